# revision 16
# baseline (speedup 1.0000x reference)
"""Trainium2 Bass kernel for CrossHeadProjection.

Math (B=1, G=1, M=32 heads, T=S=2048, I=2):
  x = inputs reshaped to (M, T, S)
  ret[n,t,s] = x[n,t,s]
             + sum_m x[m,t,s] * w[m,n]
             + sum_i qw2[t,n,i] * (sum_m qw1[t,m,i] x[m,t,s])
             + sum_i kw2[s,n,i] * (sum_m kw1[s,m,i] x[m,t,s])
             + qdd[t,n] x[n,t,s] + kdd[s,n] x[n,t,s]

Strategy: shard T across 8 cores (all terms are pointwise in t).
Everything that depends only on t (identity residual, static w, q-side
rank-2, qdd diagonal) folds into one per-t 32x32 matrix W_t, built on
host.  Four consecutive t values are packed into the 128 partitions, so
the main matmul is a block-diagonal 128x128 stationary.  The k-side
(per-s weights) cannot be a matmul; it is computed as
  y_i = x * kw1T_i                 (DVE elementwise)
  Bc_i = ones_blockdiag^T @ y_i    (PE: reduce over m + broadcast to n)
  z_i = Bc_i * kw2T_i              (DVE elementwise)
  out = A + z0 + z1 + x*kdT        (adds)
"""

import sys

if "/opt/trn_rl_repo" not in sys.path:
    sys.path.insert(0, "/opt/trn_rl_repo")

from contextlib import ExitStack

import numpy as np

import concourse.bass as bass
import concourse.tile as tile
from concourse import bacc, mybir

F32 = mybir.dt.float32
BF16 = mybir.dt.bfloat16

B, H, T, S = 1, 32, 2048, 2048
M = 32
I = 2
NCORES = 8
TC = T // NCORES  # 256 t per core
TB = 4            # t values packed per 128-partition tile
NJ = TC // TB     # 64 tiles per core
CH = 512          # psum chunk (free dim per matmul)


def build_nc(nj=NJ, s=S, ch=CH):
    """Build the per-core Bass program (identical on all cores)."""
    nc = bacc.Bacc("TRN2", target_bir_lowering=False)

    xs = nc.dram_tensor("xs", (nj, 128, s), F32, kind="ExternalInput")
    w4 = nc.dram_tensor("w4", (nj, 128, 128), F32, kind="ExternalInput")
    onesb = nc.dram_tensor("onesb", (128, 128), F32, kind="ExternalInput")
    kw1t0 = nc.dram_tensor("kw1t0", (128, s), F32, kind="ExternalInput")
    kw1t1 = nc.dram_tensor("kw1t1", (128, s), F32, kind="ExternalInput")
    kw2t0 = nc.dram_tensor("kw2t0", (128, s), F32, kind="ExternalInput")
    kw2t1 = nc.dram_tensor("kw2t1", (128, s), F32, kind="ExternalInput")
    kdt = nc.dram_tensor("kdt", (128, s), F32, kind="ExternalInput")
    out4 = nc.dram_tensor("out4", (nj, 128, s), F32, kind="ExternalOutput")

    nch = s // ch

    with tile.TileContext(nc) as tc, ExitStack() as ctx:
        wpool = ctx.enter_context(tc.tile_pool(name="weights", bufs=1))
        ones_sb = wpool.tile([128, 128], F32, tag="ones")
        nc.sync.dma_start(ones_sb[:], onesb[:])
        kw1t0_sb = wpool.tile([128, s], F32, tag="kw1t0")
        nc.sync.dma_start(kw1t0_sb[:], kw1t0[:])
        kw1t1_sb = wpool.tile([128, s], F32, tag="kw1t1")
        nc.sync.dma_start(kw1t1_sb[:], kw1t1[:])
        kw2t0_sb = wpool.tile([128, s], F32, tag="kw2t0")
        nc.sync.dma_start(kw2t0_sb[:], kw2t0[:])
        kw2t1_sb = wpool.tile([128, s], F32, tag="kw2t1")
        nc.sync.dma_start(kw2t1_sb[:], kw2t1[:])
        kdt_sb = wpool.tile([128, s], F32, tag="kdt")
        nc.sync.dma_start(kdt_sb[:], kdt[:])

        xpool = ctx.enter_context(tc.tile_pool(name="x", bufs=2))
        wstp = ctx.enter_context(tc.tile_pool(name="wst", bufs=2))
        ypool = ctx.enter_context(tc.tile_pool(name="y", bufs=2))
        zpool = ctx.enter_context(tc.tile_pool(name="z", bufs=2))
        opool = ctx.enter_context(tc.tile_pool(name="o", bufs=2))
        psA = ctx.enter_context(tc.tile_pool(name="psA", bufs=2, space="PSUM"))
        psB = ctx.enter_context(tc.tile_pool(name="psB", bufs=2, space="PSUM"))
        psC = ctx.enter_context(tc.tile_pool(name="psC", bufs=2, space="PSUM"))

        for j in range(nj):
            x4 = xpool.tile([128, s], F32, tag="x4")
            nc.sync.dma_start(x4[:], xs[j])
            wst = wstp.tile([128, 128], F32, tag="wst")
            nc.sync.dma_start(wst[:], w4[j])

            for c in range(nch):
                sl = bass.ts(c, ch)
                xc = x4[:, sl]

                A = psA.tile([128, ch], F32, tag="A")
                nc.tensor.matmul(A[:], wst[:], xc, start=True, stop=True)

                y0 = ypool.tile([128, ch], F32, tag="y0")
                nc.vector.tensor_mul(y0[:], xc, kw1t0_sb[:, sl])
                Bc0 = psB.tile([128, ch], F32, tag="B0")
                nc.tensor.matmul(Bc0[:], ones_sb[:], y0[:], start=True, stop=True)
                z0 = zpool.tile([128, ch], F32, tag="z0")
                nc.vector.tensor_mul(z0[:], Bc0[:], kw2t0_sb[:, sl])

                y1 = ypool.tile([128, ch], F32, tag="y1")
                nc.vector.tensor_mul(y1[:], xc, kw1t1_sb[:, sl])
                Bc1 = psC.tile([128, ch], F32, tag="B1")
                nc.tensor.matmul(Bc1[:], ones_sb[:], y1[:], start=True, stop=True)
                z1 = zpool.tile([128, ch], F32, tag="z1")
                nc.vector.tensor_mul(z1[:], Bc1[:], kw2t1_sb[:, sl])

                kdm = zpool.tile([128, ch], F32, tag="kdm")
                nc.vector.tensor_mul(kdm[:], xc, kdt_sb[:, sl])

                o1 = opool.tile([128, ch], F32, tag="o1")
                nc.vector.tensor_add(o1[:], A[:], z0[:])
                o2 = opool.tile([128, ch], F32, tag="o2")
                nc.vector.tensor_add(o2[:], o1[:], z1[:])
                o3 = opool.tile([128, ch], F32, tag="o3")
                nc.vector.tensor_add(o3[:], o2[:], kdm[:])

                nc.sync.dma_start(out4[j][:, sl], o3[:])

    nc.compile()
    return nc


def build_nc_v2(nj=NJ, s=S, ch=CH):
    """v2: bf16 compute of the correction term only (residual added on
    host).  Engine split: PE matmuls, DVE y0/y1/z0, GpSimd z1/kdm, ACT
    PSUM evacuation.  PSUM: A half-tiles (2 banks x2 bufs) + B0 + B1."""
    nc = bacc.Bacc("TRN2", target_bir_lowering=False)

    half = s // 2
    ch = min(ch, half)

    xs = nc.dram_tensor("xs", (nj, 128, s), BF16, kind="ExternalInput")
    w4 = nc.dram_tensor("w4", (nj, 128, 128), BF16, kind="ExternalInput")
    onesb = nc.dram_tensor("onesb", (128, 128), BF16, kind="ExternalInput")
    identb = nc.dram_tensor("identb", (128, 128), BF16, kind="ExternalInput")
    kw1t0 = nc.dram_tensor("kw1t0", (128, s), BF16, kind="ExternalInput")
    kw1t1 = nc.dram_tensor("kw1t1", (128, s), BF16, kind="ExternalInput")
    kw2t0 = nc.dram_tensor("kw2t0", (128, s), BF16, kind="ExternalInput")
    kw2t1 = nc.dram_tensor("kw2t1", (128, s), BF16, kind="ExternalInput")
    kdt = nc.dram_tensor("kdt", (128, s), BF16, kind="ExternalInput")
    out4 = nc.dram_tensor("out4", (nj, 128, s), BF16, kind="ExternalOutput")

    with tile.TileContext(nc) as tc, ExitStack() as ctx:
        wpool = ctx.enter_context(tc.tile_pool(name="weights", bufs=1))
        ones_sb = wpool.tile([128, 128], BF16, tag="ones")
        nc.sync.dma_start(ones_sb[:], onesb[:])
        ident_sb = wpool.tile([128, 128], BF16, tag="ident")
        nc.sync.dma_start(ident_sb[:], identb[:])
        kw1t0_sb = wpool.tile([128, s], BF16, tag="kw1t0")
        nc.sync.dma_start(kw1t0_sb[:], kw1t0[:])
        kw1t1_sb = wpool.tile([128, s], BF16, tag="kw1t1")
        nc.sync.dma_start(kw1t1_sb[:], kw1t1[:])
        kw2t0_sb = wpool.tile([128, s], BF16, tag="kw2t0")
        nc.sync.dma_start(kw2t0_sb[:], kw2t0[:])
        kw2t1_sb = wpool.tile([128, s], BF16, tag="kw2t1")
        nc.sync.dma_start(kw2t1_sb[:], kw2t1[:])
        kdt_sb = wpool.tile([128, s], BF16, tag="kdt")
        nc.sync.dma_start(kdt_sb[:], kdt[:])

        xpool = ctx.enter_context(tc.tile_pool(name="x", bufs=3))
        wstp = ctx.enter_context(tc.tile_pool(name="wst", bufs=2))
        ypool = ctx.enter_context(tc.tile_pool(name="y", bufs=2))
        zpool = ctx.enter_context(tc.tile_pool(name="z", bufs=2))
        bspool = ctx.enter_context(tc.tile_pool(name="bs", bufs=2))
        opool = ctx.enter_context(tc.tile_pool(name="o", bufs=2))
        psA = ctx.enter_context(tc.tile_pool(name="psA", bufs=2, space="PSUM"))
        psB0 = ctx.enter_context(tc.tile_pool(name="psB0", bufs=1, space="PSUM"))
        psB1 = ctx.enter_context(tc.tile_pool(name="psB1", bufs=1, space="PSUM"))

        for j in range(nj):
            x4 = xpool.tile([128, s], BF16, tag="x4")
            nc.sync.dma_start(x4[:], xs[j])
            wst = wstp.tile([128, 128], BF16, tag="wst")
            nc.sync.dma_start(wst[:], w4[j])

            # elementwise multiplies (full-width, SBUF only)
            y0 = ypool.tile([128, s], BF16, tag="y0")
            nc.vector.tensor_mul(y0[:], x4[:], kw1t0_sb[:])
            y1 = ypool.tile([128, s], BF16, tag="y1")
            nc.vector.tensor_mul(y1[:], x4[:], kw1t1_sb[:])
            kdm = zpool.tile([128, s], BF16, tag="kdm")
            nc.gpsimd.tensor_mul(kdm[:], x4[:], kdt_sb[:])

            Bs1 = bspool.tile([128, s], BF16, tag="bs1")
            z0 = zpool.tile([128, s], BF16, tag="z0")
            z1 = zpool.tile([128, s], BF16, tag="z1")
            o = opool.tile([128, s], BF16, tag="o")

            for h in range(2):
                hs = bass.ts(h, half)

                # k-side i=0: reduce+broadcast, then multiply from PSUM (DVE)
                B0 = psB0.tile([128, half], F32, tag="B0")
                for c in range(half // ch):
                    cs = bass.ts(h * (half // ch) + c, ch)
                    nc.tensor.matmul(
                        B0[:, bass.ts(c, ch)], ones_sb[:], y0[:, cs],
                        start=True, stop=True,
                    )
                nc.vector.tensor_mul(z0[:, hs], B0[:], kw2t0_sb[:, hs])

                # k-side i=1: reduce+broadcast, ACT evac, multiply (GpSimd)
                B1 = psB1.tile([128, half], F32, tag="B1")
                for c in range(half // ch):
                    cs = bass.ts(h * (half // ch) + c, ch)
                    nc.tensor.matmul(
                        B1[:, bass.ts(c, ch)], ones_sb[:], y1[:, cs],
                        start=True, stop=True,
                    )
                nc.scalar.copy(Bs1[:, hs], B1[:])
                nc.gpsimd.tensor_mul(z1[:, hs], Bs1[:, hs], kw2t1_sb[:, hs])

                # correction matmul + accumulate z0, z1, kdm (one psum
                # accumulation group per chunk, contiguous)
                A = psA.tile([128, half], F32, tag="A")
                for c in range(half // ch):
                    cs = bass.ts(h * (half // ch) + c, ch)
                    ca = bass.ts(c, ch)
                    nc.tensor.matmul(A[:, ca], wst[:], x4[:, cs],
                                     start=True, stop=False)
                    nc.tensor.matmul(A[:, ca], ident_sb[:], z0[:, cs],
                                     start=False, stop=False)
                    nc.tensor.matmul(A[:, ca], ident_sb[:], z1[:, cs],
                                     start=False, stop=False)
                    nc.tensor.matmul(A[:, ca], ident_sb[:], kdm[:, cs],
                                     start=False, stop=True)

                # evacuate correction to SBUF (bf16) and store
                nc.scalar.copy(o[:, hs], A[:])

            nc.sync.dma_start(out4[j], o[:])

    nc.compile()
    return nc


def prep_core_inputs(xc, w, qw1c, qw2c, kw1, kw2, qddc, kdd):
    """Build the per-core input map.

    xc:   (M, Tc, S) f32      core's t-slice of x
    w:    (M, M)
    qw1c, qw2c: (Tc, M, I)    core's t-slice
    kw1, kw2:   (S, M, I)     full (shared across cores)
    qddc: (Tc, M)
    kdd:  (S, M)
    """
    mm, tc_, s = xc.shape
    nj = tc_ // TB

    # x rearranged: [m, j, tb, s] -> [j, (tb m), s]
    xs = np.ascontiguousarray(
        xc.reshape(mm, nj, TB, s).transpose(1, 2, 0, 3).reshape(nj, TB * mm, s)
    )

    # per-t mixing matrix W_t[m, n]
    wt = np.eye(mm, dtype=np.float32)[None] + w[None].astype(np.float32)
    wt = wt + np.einsum("tmi,tni->tmn", qw1c, qw2c, dtype=np.float64).astype(np.float32)
    idx = np.arange(mm)
    wt[:, idx, idx] += qddc
    # block-diagonal packing: w4[j, (tb m), (tb n)] = wt[4j+tb, m, n]
    w4 = np.zeros((nj, TB, mm, TB, mm), dtype=np.float32)
    tb = np.arange(TB)
    w4[:, tb, :, tb, :] = wt.reshape(nj, TB, mm, mm).transpose(1, 0, 2, 3)
    w4 = w4.reshape(nj, TB * mm, TB * mm)

    onesb = np.zeros((TB, mm, TB, mm), dtype=np.float32)
    onesb[tb, :, tb, :] = 1.0
    onesb = onesb.reshape(TB * mm, TB * mm)

    def tile4(a):  # (S, M) -> (128, S)
        return np.ascontiguousarray(np.tile(a.T.astype(np.float32), (TB, 1)))

    return {
        "xs": xs.astype(np.float32),
        "w4": w4,
        "onesb": onesb,
        "kw1t0": tile4(kw1[:, :, 0]),
        "kw1t1": tile4(kw1[:, :, 1]),
        "kw2t0": tile4(kw2[:, :, 0]),
        "kw2t1": tile4(kw2[:, :, 1]),
        "kdt": tile4(kdd),
    }


def gather_core_output(out4):
    """(nj, 128, s) -> (M, Tc, s)"""
    nj, p, s = out4.shape
    return np.ascontiguousarray(
        out4.reshape(nj, TB, M, s).transpose(2, 0, 1, 3).reshape(M, nj * TB, s)
    )


def build_nc_v3(nj=NJ, s=S, ch=CH):
    """v3: no GpSimd.  DVE: y0/y1/kdm (bf16 2x) + z0/z1 (bf16 2x, from
    ACT-evacuated broadcasts).  ACT: B0/B1/A evacuations.  PE emission
    ordered for density: bcasts + mains early, ident-accumulates late,
    stationary changes grouped."""
    nc = bacc.Bacc("TRN2", target_bir_lowering=False)

    half = s // 2
    ch = min(ch, half)
    nch = half // ch

    xs = nc.dram_tensor("xs", (nj, 128, s), BF16, kind="ExternalInput")
    w4 = nc.dram_tensor("w4", (nj, 128, 128), BF16, kind="ExternalInput")
    onesb = nc.dram_tensor("onesb", (128, 128), BF16, kind="ExternalInput")
    identb = nc.dram_tensor("identb", (128, 128), BF16, kind="ExternalInput")
    kw1t0 = nc.dram_tensor("kw1t0", (128, s), BF16, kind="ExternalInput")
    kw1t1 = nc.dram_tensor("kw1t1", (128, s), BF16, kind="ExternalInput")
    kw2t0 = nc.dram_tensor("kw2t0", (128, s), BF16, kind="ExternalInput")
    kw2t1 = nc.dram_tensor("kw2t1", (128, s), BF16, kind="ExternalInput")
    kdt = nc.dram_tensor("kdt", (128, s), BF16, kind="ExternalInput")
    out4 = nc.dram_tensor("out4", (nj, 128, s), BF16, kind="ExternalOutput")

    with tile.TileContext(nc) as tc, ExitStack() as ctx:
        wpool = ctx.enter_context(tc.tile_pool(name="weights", bufs=1))
        ones_sb = wpool.tile([128, 128], BF16, tag="ones")
        nc.sync.dma_start(ones_sb[:], onesb[:])
        ident_sb = wpool.tile([128, 128], BF16, tag="ident")
        nc.sync.dma_start(ident_sb[:], identb[:])
        kw1t0_sb = wpool.tile([128, s], BF16, tag="kw1t0")
        nc.sync.dma_start(kw1t0_sb[:], kw1t0[:])
        kw1t1_sb = wpool.tile([128, s], BF16, tag="kw1t1")
        nc.sync.dma_start(kw1t1_sb[:], kw1t1[:])
        kw2t0_sb = wpool.tile([128, s], BF16, tag="kw2t0")
        nc.sync.dma_start(kw2t0_sb[:], kw2t0[:])
        kw2t1_sb = wpool.tile([128, s], BF16, tag="kw2t1")
        nc.sync.dma_start(kw2t1_sb[:], kw2t1[:])
        kdt_sb = wpool.tile([128, s], BF16, tag="kdt")
        nc.sync.dma_start(kdt_sb[:], kdt[:])

        xpool = ctx.enter_context(tc.tile_pool(name="x", bufs=3))
        wstp = ctx.enter_context(tc.tile_pool(name="wst", bufs=3))
        ypool = ctx.enter_context(tc.tile_pool(name="y", bufs=2))
        zpool = ctx.enter_context(tc.tile_pool(name="z", bufs=2))
        bspool = ctx.enter_context(tc.tile_pool(name="bs", bufs=2))
        opool = ctx.enter_context(tc.tile_pool(name="o", bufs=2))
        psA = ctx.enter_context(tc.tile_pool(name="psA", bufs=2, space="PSUM"))
        psB0 = ctx.enter_context(tc.tile_pool(name="psB0", bufs=1, space="PSUM"))
        psB1 = ctx.enter_context(tc.tile_pool(name="psB1", bufs=1, space="PSUM"))

        for j in range(nj):
            x4 = xpool.tile([128, s], BF16, tag="x4")
            nc.sync.dma_start(x4[:], xs[j])
            wst = wstp.tile([128, 128], BF16, tag="wst")
            nc.sync.dma_start(wst[:], w4[j])

            # SBUF-only elementwise multiplies (DVE, bf16 2x)
            y0 = ypool.tile([128, s], BF16, tag="y0")
            nc.vector.tensor_mul(y0[:], x4[:], kw1t0_sb[:])
            y1 = ypool.tile([128, s], BF16, tag="y1")
            nc.vector.tensor_mul(y1[:], x4[:], kw1t1_sb[:])
            kdm = zpool.tile([128, s], BF16, tag="kdm")
            nc.vector.tensor_mul(kdm[:], x4[:], kdt_sb[:])

            Bs0 = bspool.tile([128, s], BF16, tag="bs0")
            Bs1 = bspool.tile([128, s], BF16, tag="bs1")
            z0 = zpool.tile([128, s], BF16, tag="z0")
            z1 = zpool.tile([128, s], BF16, tag="z1")
            o = opool.tile([128, s], BF16, tag="o")

            As = []
            for h in range(2):
                hs = bass.ts(h, half)

                # reduce+broadcast matmuls (stationary: ones)
                B0 = psB0.tile([128, half], F32, tag="B0")
                for c in range(nch):
                    cs = bass.ts(h * nch + c, ch)
                    nc.tensor.matmul(B0[:, bass.ts(c, ch)], ones_sb[:],
                                     y0[:, cs], start=True, stop=True)
                B1 = psB1.tile([128, half], F32, tag="B1")
                for c in range(nch):
                    cs = bass.ts(h * nch + c, ch)
                    nc.tensor.matmul(B1[:, bass.ts(c, ch)], ones_sb[:],
                                     y1[:, cs], start=True, stop=True)

                # evacuate broadcasts (ACT), then multiply by kw2 (DVE)
                nc.scalar.copy(Bs0[:, hs], B0[:])
                nc.scalar.copy(Bs1[:, hs], B1[:])
                nc.vector.tensor_mul(z0[:, hs], Bs0[:, hs], kw2t0_sb[:, hs])
                nc.vector.tensor_mul(z1[:, hs], Bs1[:, hs], kw2t1_sb[:, hs])

                # correction matmuls (stationary: wst) open the psum groups
                A = psA.tile([128, half], F32, tag="A")
                As.append(A)
                for c in range(nch):
                    cs = bass.ts(h * nch + c, ch)
                    nc.tensor.matmul(A[:, bass.ts(c, ch)], wst[:], x4[:, cs],
                                     start=True, stop=False)

            # identity-accumulates (stationary: ident), grouped late so z's
            # are ready; banks interleave but each bank's group is ordered
            for h in range(2):
                A = As[h]
                for zt in (z0, z1):
                    for c in range(nch):
                        cs = bass.ts(h * nch + c, ch)
                        nc.tensor.matmul(A[:, bass.ts(c, ch)], ident_sb[:],
                                         zt[:, cs], start=False, stop=False)
                for c in range(nch):
                    cs = bass.ts(h * nch + c, ch)
                    nc.tensor.matmul(A[:, bass.ts(c, ch)], ident_sb[:],
                                     kdm[:, cs], start=False, stop=True)
                nc.scalar.copy(o[:, bass.ts(h, half)], A[:])

            nc.sync.dma_start(out4[j], o[:])

    nc.compile()
    return nc


def build_nc_v4(nj=NJ, s=S, ch=CH):
    """v4: like v3 plus (a) j-pairs share one (128, 2s) x tile so the
    y0/y1/kdm DVE multiplies run at double width (less per-op overhead),
    (b) B0|B1 share one 4-bank psum tile so each half needs a single
    2048-wide ACT evacuation instead of two."""
    nc = bacc.Bacc("TRN2", target_bir_lowering=False)

    half = s // 2
    ch = min(ch, half)
    nch = half // ch
    assert nj % 2 == 0

    xs = nc.dram_tensor("xs", (nj, 128, s), BF16, kind="ExternalInput")
    w4 = nc.dram_tensor("w4", (nj, 128, 128), BF16, kind="ExternalInput")
    onesb = nc.dram_tensor("onesb", (128, 128), BF16, kind="ExternalInput")
    identb = nc.dram_tensor("identb", (128, 128), BF16, kind="ExternalInput")
    kw1t0 = nc.dram_tensor("kw1t0", (128, s), BF16, kind="ExternalInput")
    kw1t1 = nc.dram_tensor("kw1t1", (128, s), BF16, kind="ExternalInput")
    kw2t0 = nc.dram_tensor("kw2t0", (128, s), BF16, kind="ExternalInput")
    kw2t1 = nc.dram_tensor("kw2t1", (128, s), BF16, kind="ExternalInput")
    kdt = nc.dram_tensor("kdt", (128, s), BF16, kind="ExternalInput")
    out4 = nc.dram_tensor("out4", (nj, 128, s), BF16, kind="ExternalOutput")

    with tile.TileContext(nc) as tc, ExitStack() as ctx:
        # persistent weights; kw1/kdt doubled along free for 2s-wide ops
        wpool = ctx.enter_context(tc.tile_pool(name="weights", bufs=1))
        ones_sb = wpool.tile([128, 128], BF16, tag="ones")
        nc.sync.dma_start(ones_sb[:], onesb[:])
        ident_sb = wpool.tile([128, 128], BF16, tag="ident")
        nc.sync.dma_start(ident_sb[:], identb[:])
        kw1t0_sb = wpool.tile([128, 2 * s], BF16, tag="kw1t0")
        nc.sync.dma_start(kw1t0_sb[:, 0:s], kw1t0[:])
        nc.sync.dma_start(kw1t0_sb[:, s:2 * s], kw1t0[:])
        kw1t1_sb = wpool.tile([128, 2 * s], BF16, tag="kw1t1")
        nc.sync.dma_start(kw1t1_sb[:, 0:s], kw1t1[:])
        nc.sync.dma_start(kw1t1_sb[:, s:2 * s], kw1t1[:])
        kdt_sb = wpool.tile([128, 2 * s], BF16, tag="kdt")
        nc.sync.dma_start(kdt_sb[:, 0:s], kdt[:])
        nc.sync.dma_start(kdt_sb[:, s:2 * s], kdt[:])
        kw2t0_sb = wpool.tile([128, s], BF16, tag="kw2t0")
        nc.sync.dma_start(kw2t0_sb[:], kw2t0[:])
        kw2t1_sb = wpool.tile([128, s], BF16, tag="kw2t1")
        nc.sync.dma_start(kw2t1_sb[:], kw2t1[:])

        xpool = ctx.enter_context(tc.tile_pool(name="x", bufs=2))
        wstp = ctx.enter_context(tc.tile_pool(name="wst", bufs=2))
        ypool = ctx.enter_context(tc.tile_pool(name="y", bufs=2))
        kpool = ctx.enter_context(tc.tile_pool(name="k", bufs=2))
        zpool = ctx.enter_context(tc.tile_pool(name="z", bufs=2))
        bspool = ctx.enter_context(tc.tile_pool(name="bs", bufs=3))
        opool = ctx.enter_context(tc.tile_pool(name="o", bufs=2))
        psA = ctx.enter_context(tc.tile_pool(name="psA", bufs=2, space="PSUM"))
        psB = ctx.enter_context(tc.tile_pool(name="psB", bufs=1, space="PSUM"))

        for jp in range(nj // 2):
            j0 = 2 * jp
            x2 = xpool.tile([128, 2 * s], BF16, tag="x2")
            nc.sync.dma_start(x2[:, 0:s], xs[j0])
            nc.sync.dma_start(x2[:, s:2 * s], xs[j0 + 1])
            wst2 = wstp.tile([128, 256], BF16, tag="wst2")
            nc.sync.dma_start(wst2[:, 0:128], w4[j0])
            nc.sync.dma_start(wst2[:, 128:256], w4[j0 + 1])

            # double-width elementwise multiplies (DVE bf16 2x)
            y0 = ypool.tile([128, 2 * s], BF16, tag="y0")
            nc.vector.tensor_mul(y0[:], x2[:], kw1t0_sb[:])
            y1 = ypool.tile([128, 2 * s], BF16, tag="y1")
            nc.vector.tensor_mul(y1[:], x2[:], kw1t1_sb[:])
            kdm = kpool.tile([128, 2 * s], BF16, tag="kdm")
            nc.vector.tensor_mul(kdm[:], x2[:], kdt_sb[:])

            o = opool.tile([128, 2 * s], BF16, tag="o")

            for jj in range(2):
                xoff = jj * s
                z0 = zpool.tile([128, s], BF16, tag="z0")
                z1 = zpool.tile([128, s], BF16, tag="z1")

                As = []
                for h in range(2):
                    hoff = xoff + h * half
                    hs = bass.ts(h, half)

                    # reduce+broadcast for both i into one 4-bank psum tile
                    Bb = psB.tile([128, 2 * half], F32, tag="B")
                    for c in range(nch):
                        nc.tensor.matmul(
                            Bb[:, bass.ts(c, ch)], ones_sb[:],
                            y0[:, hoff:hoff + half][:, bass.ts(c, ch)],
                            start=True, stop=True)
                    for c in range(nch):
                        nc.tensor.matmul(
                            Bb[:, bass.ts(nch + c, ch)], ones_sb[:],
                            y1[:, hoff:hoff + half][:, bass.ts(c, ch)],
                            start=True, stop=True)

                    # one wide evac of [B0h | B1h], then the two z mults
                    Bs = bspool.tile([128, 2 * half], BF16, tag="bs")
                    nc.scalar.copy(Bs[:], Bb[:])
                    nc.vector.tensor_mul(z0[:, hs], Bs[:, 0:half],
                                         kw2t0_sb[:, hs])
                    nc.vector.tensor_mul(z1[:, hs], Bs[:, half:2 * half],
                                         kw2t1_sb[:, hs])

                    # correction matmuls open the A psum groups
                    A = psA.tile([128, half], F32, tag="A")
                    As.append(A)
                    for c in range(nch):
                        nc.tensor.matmul(
                            A[:, bass.ts(c, ch)], wst2[:, bass.ts(jj, 128)],
                            x2[:, hoff:hoff + half][:, bass.ts(c, ch)],
                            start=True, stop=False)

                # identity-accumulates, then evacuate
                for h in range(2):
                    hoff = xoff + h * half
                    A = As[h]
                    for zt in (z0, z1):
                        for c in range(nch):
                            nc.tensor.matmul(
                                A[:, bass.ts(c, ch)], ident_sb[:],
                                zt[:, h * half:(h + 1) * half][:, bass.ts(c, ch)],
                                start=False, stop=False)
                    for c in range(nch):
                        nc.tensor.matmul(
                            A[:, bass.ts(c, ch)], ident_sb[:],
                            kdm[:, hoff:hoff + half][:, bass.ts(c, ch)],
                            start=False, stop=True)
                    nc.scalar.copy(o[:, hoff:hoff + half], A[:])

            nc.sync.dma_start(out4[j0], o[:, 0:s])
            nc.sync.dma_start(out4[j0 + 1], o[:, s:2 * s])

    nc.compile()
    return nc


def prep_core_inputs_v2(xc, w, qw1c, qw2c, kw1, kw2, qddc, kdd):
    """v2 per-core input map: bf16, correction-only W' (no identity)."""
    import ml_dtypes

    bf16 = ml_dtypes.bfloat16
    mm, tc_, s = xc.shape
    nj = tc_ // TB

    xs = np.ascontiguousarray(
        xc.reshape(mm, nj, TB, s).transpose(1, 2, 0, 3).reshape(nj, TB * mm, s)
    ).astype(bf16)

    # per-t correction matrix W'_t[m, n] (no identity)
    wt = np.broadcast_to(w.astype(np.float32), (tc_, mm, mm)).copy()
    wt += np.einsum("tmi,tni->tmn", qw1c, qw2c, dtype=np.float64).astype(np.float32)
    idx = np.arange(mm)
    wt[:, idx, idx] += qddc
    w4 = np.zeros((nj, TB, mm, TB, mm), dtype=np.float32)
    tb = np.arange(TB)
    w4[:, tb, :, tb, :] = wt.reshape(nj, TB, mm, mm).transpose(1, 0, 2, 3)
    w4 = w4.reshape(nj, TB * mm, TB * mm).astype(bf16)

    onesb = np.zeros((TB, mm, TB, mm), dtype=np.float32)
    onesb[tb, :, tb, :] = 1.0
    onesb = onesb.reshape(TB * mm, TB * mm).astype(bf16)
    identb = np.eye(TB * mm, dtype=np.float32).astype(bf16)

    def tile4(a):  # (S, M) -> (128, S) bf16
        return np.ascontiguousarray(np.tile(a.T.astype(np.float32), (TB, 1))).astype(bf16)

    return {
        "xs": xs,
        "w4": w4,
        "onesb": onesb,
        "identb": identb,
        "kw1t0": tile4(kw1[:, :, 0]),
        "kw1t1": tile4(kw1[:, :, 1]),
        "kw2t0": tile4(kw2[:, :, 0]),
        "kw2t1": tile4(kw2[:, :, 1]),
        "kdt": tile4(kdd),
    }


_NC_CACHE = {}

# test-harness knobs (not used by the grading path)
TRACE = False
TRACE_DIR = None
LAST_RES = None
VERSION = 4


def _get_nc():
    key = f"nc_v{VERSION}"
    if key not in _NC_CACHE:
        builders = {1: build_nc, 2: build_nc_v2, 3: build_nc_v3, 4: build_nc_v4}
        _NC_CACHE[key] = builders[VERSION]()
    return _NC_CACHE[key]


def kernel(inputs, w, qw1, qw2, kw1, kw2, qdd, kdd):
    from concourse.bass_utils import run_bass_kernel_spmd

    x = np.asarray(inputs)          # (1, 32, T, S)
    w_ = np.asarray(w)[0]           # (M, M)
    qw1_ = np.asarray(qw1)[0, :, 0]  # (T, M, I)
    qw2_ = np.asarray(qw2)[0, :, 0]
    kw1_ = np.asarray(kw1)[0, :, 0]  # (S, M, I)
    kw2_ = np.asarray(kw2)[0, :, 0]
    qdd_ = np.asarray(qdd)[0, :, 0]  # (T, M)
    kdd_ = np.asarray(kdd)[0, :, 0]  # (S, M)

    xm = x[0]  # (M, T, S)

    prep = prep_core_inputs_v2 if VERSION >= 2 else prep_core_inputs
    in_maps = []
    for c in range(NCORES):
        tsl = slice(c * TC, (c + 1) * TC)
        in_maps.append(
            prep(
                xm[:, tsl, :], w_, qw1_[tsl], qw2_[tsl], kw1_, kw2_,
                qdd_[tsl], kdd_,
            )
        )

    nc = _get_nc()
    kwargs = {}
    if TRACE:
        kwargs = {"trace": True, "tmpdir": TRACE_DIR}
    res = run_bass_kernel_spmd(nc, in_maps, list(range(NCORES)), **kwargs)
    global LAST_RES
    LAST_RES = res

    out = np.empty((M, T, S), dtype=np.float32)
    for c in range(NCORES):
        corr = gather_core_output(np.asarray(res.results[c]["out4"], dtype=np.float32))
        out[:, c * TC:(c + 1) * TC, :] = corr
    if VERSION >= 2:
        out += xm  # exact fp32 residual
    return out.reshape(B, H, T, S)


# revision 17
# speedup vs baseline: 1.2698x; 1.2698x over previous
"""Trainium2 Bass kernel for CrossHeadProjection.

Math (B=1, G=1, M=32 heads, T=S=2048, I=2):
  x = inputs reshaped to (M, T, S)
  ret[n,t,s] = x[n,t,s]
             + sum_m x[m,t,s] * w[m,n]
             + sum_i qw2[t,n,i] * (sum_m qw1[t,m,i] x[m,t,s])
             + sum_i kw2[s,n,i] * (sum_m kw1[s,m,i] x[m,t,s])
             + qdd[t,n] x[n,t,s] + kdd[s,n] x[n,t,s]

Strategy: shard T across 8 cores (all terms are pointwise in t).
Everything that depends only on t (identity residual, static w, q-side
rank-2, qdd diagonal) folds into one per-t 32x32 matrix W_t, built on
host.  Four consecutive t values are packed into the 128 partitions, so
the main matmul is a block-diagonal 128x128 stationary.  The k-side
(per-s weights) cannot be a matmul; it is computed as
  y_i = x * kw1T_i                 (DVE elementwise)
  Bc_i = ones_blockdiag^T @ y_i    (PE: reduce over m + broadcast to n)
  z_i = Bc_i * kw2T_i              (DVE elementwise)
  out = A + z0 + z1 + x*kdT        (adds)
"""

import sys

if "/opt/trn_rl_repo" not in sys.path:
    sys.path.insert(0, "/opt/trn_rl_repo")

from contextlib import ExitStack

import numpy as np

import concourse.bass as bass
import concourse.tile as tile
from concourse import bacc, mybir

F32 = mybir.dt.float32
BF16 = mybir.dt.bfloat16

B, H, T, S = 1, 32, 2048, 2048
M = 32
I = 2
NCORES = 8
TC = T // NCORES  # 256 t per core
TB = 4            # t values packed per 128-partition tile
NJ = TC // TB     # 64 tiles per core
CH = 512          # psum chunk (free dim per matmul)


def build_nc(nj=NJ, s=S, ch=CH):
    """Build the per-core Bass program (identical on all cores)."""
    nc = bacc.Bacc("TRN2", target_bir_lowering=False)

    xs = nc.dram_tensor("xs", (nj, 128, s), F32, kind="ExternalInput")
    w4 = nc.dram_tensor("w4", (nj, 128, 128), F32, kind="ExternalInput")
    onesb = nc.dram_tensor("onesb", (128, 128), F32, kind="ExternalInput")
    kw1t0 = nc.dram_tensor("kw1t0", (128, s), F32, kind="ExternalInput")
    kw1t1 = nc.dram_tensor("kw1t1", (128, s), F32, kind="ExternalInput")
    kw2t0 = nc.dram_tensor("kw2t0", (128, s), F32, kind="ExternalInput")
    kw2t1 = nc.dram_tensor("kw2t1", (128, s), F32, kind="ExternalInput")
    kdt = nc.dram_tensor("kdt", (128, s), F32, kind="ExternalInput")
    out4 = nc.dram_tensor("out4", (nj, 128, s), F32, kind="ExternalOutput")

    nch = s // ch

    with tile.TileContext(nc) as tc, ExitStack() as ctx:
        wpool = ctx.enter_context(tc.tile_pool(name="weights", bufs=1))
        ones_sb = wpool.tile([128, 128], F32, tag="ones")
        nc.sync.dma_start(ones_sb[:], onesb[:])
        kw1t0_sb = wpool.tile([128, s], F32, tag="kw1t0")
        nc.sync.dma_start(kw1t0_sb[:], kw1t0[:])
        kw1t1_sb = wpool.tile([128, s], F32, tag="kw1t1")
        nc.sync.dma_start(kw1t1_sb[:], kw1t1[:])
        kw2t0_sb = wpool.tile([128, s], F32, tag="kw2t0")
        nc.sync.dma_start(kw2t0_sb[:], kw2t0[:])
        kw2t1_sb = wpool.tile([128, s], F32, tag="kw2t1")
        nc.sync.dma_start(kw2t1_sb[:], kw2t1[:])
        kdt_sb = wpool.tile([128, s], F32, tag="kdt")
        nc.sync.dma_start(kdt_sb[:], kdt[:])

        xpool = ctx.enter_context(tc.tile_pool(name="x", bufs=2))
        wstp = ctx.enter_context(tc.tile_pool(name="wst", bufs=2))
        ypool = ctx.enter_context(tc.tile_pool(name="y", bufs=2))
        zpool = ctx.enter_context(tc.tile_pool(name="z", bufs=2))
        opool = ctx.enter_context(tc.tile_pool(name="o", bufs=2))
        psA = ctx.enter_context(tc.tile_pool(name="psA", bufs=2, space="PSUM"))
        psB = ctx.enter_context(tc.tile_pool(name="psB", bufs=2, space="PSUM"))
        psC = ctx.enter_context(tc.tile_pool(name="psC", bufs=2, space="PSUM"))

        for j in range(nj):
            x4 = xpool.tile([128, s], F32, tag="x4")
            nc.sync.dma_start(x4[:], xs[j])
            wst = wstp.tile([128, 128], F32, tag="wst")
            nc.sync.dma_start(wst[:], w4[j])

            for c in range(nch):
                sl = bass.ts(c, ch)
                xc = x4[:, sl]

                A = psA.tile([128, ch], F32, tag="A")
                nc.tensor.matmul(A[:], wst[:], xc, start=True, stop=True)

                y0 = ypool.tile([128, ch], F32, tag="y0")
                nc.vector.tensor_mul(y0[:], xc, kw1t0_sb[:, sl])
                Bc0 = psB.tile([128, ch], F32, tag="B0")
                nc.tensor.matmul(Bc0[:], ones_sb[:], y0[:], start=True, stop=True)
                z0 = zpool.tile([128, ch], F32, tag="z0")
                nc.vector.tensor_mul(z0[:], Bc0[:], kw2t0_sb[:, sl])

                y1 = ypool.tile([128, ch], F32, tag="y1")
                nc.vector.tensor_mul(y1[:], xc, kw1t1_sb[:, sl])
                Bc1 = psC.tile([128, ch], F32, tag="B1")
                nc.tensor.matmul(Bc1[:], ones_sb[:], y1[:], start=True, stop=True)
                z1 = zpool.tile([128, ch], F32, tag="z1")
                nc.vector.tensor_mul(z1[:], Bc1[:], kw2t1_sb[:, sl])

                kdm = zpool.tile([128, ch], F32, tag="kdm")
                nc.vector.tensor_mul(kdm[:], xc, kdt_sb[:, sl])

                o1 = opool.tile([128, ch], F32, tag="o1")
                nc.vector.tensor_add(o1[:], A[:], z0[:])
                o2 = opool.tile([128, ch], F32, tag="o2")
                nc.vector.tensor_add(o2[:], o1[:], z1[:])
                o3 = opool.tile([128, ch], F32, tag="o3")
                nc.vector.tensor_add(o3[:], o2[:], kdm[:])

                nc.sync.dma_start(out4[j][:, sl], o3[:])

    nc.compile()
    return nc


def build_nc_v2(nj=NJ, s=S, ch=CH):
    """v2: bf16 compute of the correction term only (residual added on
    host).  Engine split: PE matmuls, DVE y0/y1/z0, GpSimd z1/kdm, ACT
    PSUM evacuation.  PSUM: A half-tiles (2 banks x2 bufs) + B0 + B1."""
    nc = bacc.Bacc("TRN2", target_bir_lowering=False)

    half = s // 2
    ch = min(ch, half)

    xs = nc.dram_tensor("xs", (nj, 128, s), BF16, kind="ExternalInput")
    w4 = nc.dram_tensor("w4", (nj, 128, 128), BF16, kind="ExternalInput")
    onesb = nc.dram_tensor("onesb", (128, 128), BF16, kind="ExternalInput")
    identb = nc.dram_tensor("identb", (128, 128), BF16, kind="ExternalInput")
    kw1t0 = nc.dram_tensor("kw1t0", (128, s), BF16, kind="ExternalInput")
    kw1t1 = nc.dram_tensor("kw1t1", (128, s), BF16, kind="ExternalInput")
    kw2t0 = nc.dram_tensor("kw2t0", (128, s), BF16, kind="ExternalInput")
    kw2t1 = nc.dram_tensor("kw2t1", (128, s), BF16, kind="ExternalInput")
    kdt = nc.dram_tensor("kdt", (128, s), BF16, kind="ExternalInput")
    out4 = nc.dram_tensor("out4", (nj, 128, s), BF16, kind="ExternalOutput")

    with tile.TileContext(nc) as tc, ExitStack() as ctx:
        wpool = ctx.enter_context(tc.tile_pool(name="weights", bufs=1))
        ones_sb = wpool.tile([128, 128], BF16, tag="ones")
        nc.sync.dma_start(ones_sb[:], onesb[:])
        ident_sb = wpool.tile([128, 128], BF16, tag="ident")
        nc.sync.dma_start(ident_sb[:], identb[:])
        kw1t0_sb = wpool.tile([128, s], BF16, tag="kw1t0")
        nc.sync.dma_start(kw1t0_sb[:], kw1t0[:])
        kw1t1_sb = wpool.tile([128, s], BF16, tag="kw1t1")
        nc.sync.dma_start(kw1t1_sb[:], kw1t1[:])
        kw2t0_sb = wpool.tile([128, s], BF16, tag="kw2t0")
        nc.sync.dma_start(kw2t0_sb[:], kw2t0[:])
        kw2t1_sb = wpool.tile([128, s], BF16, tag="kw2t1")
        nc.sync.dma_start(kw2t1_sb[:], kw2t1[:])
        kdt_sb = wpool.tile([128, s], BF16, tag="kdt")
        nc.sync.dma_start(kdt_sb[:], kdt[:])

        xpool = ctx.enter_context(tc.tile_pool(name="x", bufs=3))
        wstp = ctx.enter_context(tc.tile_pool(name="wst", bufs=2))
        ypool = ctx.enter_context(tc.tile_pool(name="y", bufs=2))
        zpool = ctx.enter_context(tc.tile_pool(name="z", bufs=2))
        bspool = ctx.enter_context(tc.tile_pool(name="bs", bufs=2))
        opool = ctx.enter_context(tc.tile_pool(name="o", bufs=2))
        psA = ctx.enter_context(tc.tile_pool(name="psA", bufs=2, space="PSUM"))
        psB0 = ctx.enter_context(tc.tile_pool(name="psB0", bufs=1, space="PSUM"))
        psB1 = ctx.enter_context(tc.tile_pool(name="psB1", bufs=1, space="PSUM"))

        for j in range(nj):
            x4 = xpool.tile([128, s], BF16, tag="x4")
            nc.sync.dma_start(x4[:], xs[j])
            wst = wstp.tile([128, 128], BF16, tag="wst")
            nc.sync.dma_start(wst[:], w4[j])

            # elementwise multiplies (full-width, SBUF only)
            y0 = ypool.tile([128, s], BF16, tag="y0")
            nc.vector.tensor_mul(y0[:], x4[:], kw1t0_sb[:])
            y1 = ypool.tile([128, s], BF16, tag="y1")
            nc.vector.tensor_mul(y1[:], x4[:], kw1t1_sb[:])
            kdm = zpool.tile([128, s], BF16, tag="kdm")
            nc.gpsimd.tensor_mul(kdm[:], x4[:], kdt_sb[:])

            Bs1 = bspool.tile([128, s], BF16, tag="bs1")
            z0 = zpool.tile([128, s], BF16, tag="z0")
            z1 = zpool.tile([128, s], BF16, tag="z1")
            o = opool.tile([128, s], BF16, tag="o")

            for h in range(2):
                hs = bass.ts(h, half)

                # k-side i=0: reduce+broadcast, then multiply from PSUM (DVE)
                B0 = psB0.tile([128, half], F32, tag="B0")
                for c in range(half // ch):
                    cs = bass.ts(h * (half // ch) + c, ch)
                    nc.tensor.matmul(
                        B0[:, bass.ts(c, ch)], ones_sb[:], y0[:, cs],
                        start=True, stop=True,
                    )
                nc.vector.tensor_mul(z0[:, hs], B0[:], kw2t0_sb[:, hs])

                # k-side i=1: reduce+broadcast, ACT evac, multiply (GpSimd)
                B1 = psB1.tile([128, half], F32, tag="B1")
                for c in range(half // ch):
                    cs = bass.ts(h * (half // ch) + c, ch)
                    nc.tensor.matmul(
                        B1[:, bass.ts(c, ch)], ones_sb[:], y1[:, cs],
                        start=True, stop=True,
                    )
                nc.scalar.copy(Bs1[:, hs], B1[:])
                nc.gpsimd.tensor_mul(z1[:, hs], Bs1[:, hs], kw2t1_sb[:, hs])

                # correction matmul + accumulate z0, z1, kdm (one psum
                # accumulation group per chunk, contiguous)
                A = psA.tile([128, half], F32, tag="A")
                for c in range(half // ch):
                    cs = bass.ts(h * (half // ch) + c, ch)
                    ca = bass.ts(c, ch)
                    nc.tensor.matmul(A[:, ca], wst[:], x4[:, cs],
                                     start=True, stop=False)
                    nc.tensor.matmul(A[:, ca], ident_sb[:], z0[:, cs],
                                     start=False, stop=False)
                    nc.tensor.matmul(A[:, ca], ident_sb[:], z1[:, cs],
                                     start=False, stop=False)
                    nc.tensor.matmul(A[:, ca], ident_sb[:], kdm[:, cs],
                                     start=False, stop=True)

                # evacuate correction to SBUF (bf16) and store
                nc.scalar.copy(o[:, hs], A[:])

            nc.sync.dma_start(out4[j], o[:])

    nc.compile()
    return nc


def prep_core_inputs(xc, w, qw1c, qw2c, kw1, kw2, qddc, kdd):
    """Build the per-core input map.

    xc:   (M, Tc, S) f32      core's t-slice of x
    w:    (M, M)
    qw1c, qw2c: (Tc, M, I)    core's t-slice
    kw1, kw2:   (S, M, I)     full (shared across cores)
    qddc: (Tc, M)
    kdd:  (S, M)
    """
    mm, tc_, s = xc.shape
    nj = tc_ // TB

    # x rearranged: [m, j, tb, s] -> [j, (tb m), s]
    xs = np.ascontiguousarray(
        xc.reshape(mm, nj, TB, s).transpose(1, 2, 0, 3).reshape(nj, TB * mm, s)
    )

    # per-t mixing matrix W_t[m, n]
    wt = np.eye(mm, dtype=np.float32)[None] + w[None].astype(np.float32)
    wt = wt + np.einsum("tmi,tni->tmn", qw1c, qw2c, dtype=np.float64).astype(np.float32)
    idx = np.arange(mm)
    wt[:, idx, idx] += qddc
    # block-diagonal packing: w4[j, (tb m), (tb n)] = wt[4j+tb, m, n]
    w4 = np.zeros((nj, TB, mm, TB, mm), dtype=np.float32)
    tb = np.arange(TB)
    w4[:, tb, :, tb, :] = wt.reshape(nj, TB, mm, mm).transpose(1, 0, 2, 3)
    w4 = w4.reshape(nj, TB * mm, TB * mm)

    onesb = np.zeros((TB, mm, TB, mm), dtype=np.float32)
    onesb[tb, :, tb, :] = 1.0
    onesb = onesb.reshape(TB * mm, TB * mm)

    def tile4(a):  # (S, M) -> (128, S)
        return np.ascontiguousarray(np.tile(a.T.astype(np.float32), (TB, 1)))

    return {
        "xs": xs.astype(np.float32),
        "w4": w4,
        "onesb": onesb,
        "kw1t0": tile4(kw1[:, :, 0]),
        "kw1t1": tile4(kw1[:, :, 1]),
        "kw2t0": tile4(kw2[:, :, 0]),
        "kw2t1": tile4(kw2[:, :, 1]),
        "kdt": tile4(kdd),
    }


def gather_core_output(out4):
    """(nj, 128, s) -> (M, Tc, s)"""
    nj, p, s = out4.shape
    return np.ascontiguousarray(
        out4.reshape(nj, TB, M, s).transpose(2, 0, 1, 3).reshape(M, nj * TB, s)
    )


def build_nc_v3(nj=NJ, s=S, ch=CH):
    """v3: no GpSimd.  DVE: y0/y1/kdm (bf16 2x) + z0/z1 (bf16 2x, from
    ACT-evacuated broadcasts).  ACT: B0/B1/A evacuations.  PE emission
    ordered for density: bcasts + mains early, ident-accumulates late,
    stationary changes grouped."""
    nc = bacc.Bacc("TRN2", target_bir_lowering=False)

    half = s // 2
    ch = min(ch, half)
    nch = half // ch

    xs = nc.dram_tensor("xs", (nj, 128, s), BF16, kind="ExternalInput")
    w4 = nc.dram_tensor("w4", (nj, 128, 128), BF16, kind="ExternalInput")
    onesb = nc.dram_tensor("onesb", (128, 128), BF16, kind="ExternalInput")
    identb = nc.dram_tensor("identb", (128, 128), BF16, kind="ExternalInput")
    kw1t0 = nc.dram_tensor("kw1t0", (128, s), BF16, kind="ExternalInput")
    kw1t1 = nc.dram_tensor("kw1t1", (128, s), BF16, kind="ExternalInput")
    kw2t0 = nc.dram_tensor("kw2t0", (128, s), BF16, kind="ExternalInput")
    kw2t1 = nc.dram_tensor("kw2t1", (128, s), BF16, kind="ExternalInput")
    kdt = nc.dram_tensor("kdt", (128, s), BF16, kind="ExternalInput")
    out4 = nc.dram_tensor("out4", (nj, 128, s), BF16, kind="ExternalOutput")

    with tile.TileContext(nc) as tc, ExitStack() as ctx:
        wpool = ctx.enter_context(tc.tile_pool(name="weights", bufs=1))
        ones_sb = wpool.tile([128, 128], BF16, tag="ones")
        nc.sync.dma_start(ones_sb[:], onesb[:])
        ident_sb = wpool.tile([128, 128], BF16, tag="ident")
        nc.sync.dma_start(ident_sb[:], identb[:])
        kw1t0_sb = wpool.tile([128, s], BF16, tag="kw1t0")
        nc.sync.dma_start(kw1t0_sb[:], kw1t0[:])
        kw1t1_sb = wpool.tile([128, s], BF16, tag="kw1t1")
        nc.sync.dma_start(kw1t1_sb[:], kw1t1[:])
        kw2t0_sb = wpool.tile([128, s], BF16, tag="kw2t0")
        nc.sync.dma_start(kw2t0_sb[:], kw2t0[:])
        kw2t1_sb = wpool.tile([128, s], BF16, tag="kw2t1")
        nc.sync.dma_start(kw2t1_sb[:], kw2t1[:])
        kdt_sb = wpool.tile([128, s], BF16, tag="kdt")
        nc.sync.dma_start(kdt_sb[:], kdt[:])

        xpool = ctx.enter_context(tc.tile_pool(name="x", bufs=3))
        wstp = ctx.enter_context(tc.tile_pool(name="wst", bufs=3))
        ypool = ctx.enter_context(tc.tile_pool(name="y", bufs=2))
        zpool = ctx.enter_context(tc.tile_pool(name="z", bufs=2))
        bspool = ctx.enter_context(tc.tile_pool(name="bs", bufs=2))
        opool = ctx.enter_context(tc.tile_pool(name="o", bufs=2))
        psA = ctx.enter_context(tc.tile_pool(name="psA", bufs=2, space="PSUM"))
        psB0 = ctx.enter_context(tc.tile_pool(name="psB0", bufs=1, space="PSUM"))
        psB1 = ctx.enter_context(tc.tile_pool(name="psB1", bufs=1, space="PSUM"))

        for j in range(nj):
            x4 = xpool.tile([128, s], BF16, tag="x4")
            nc.sync.dma_start(x4[:], xs[j])
            wst = wstp.tile([128, 128], BF16, tag="wst")
            nc.sync.dma_start(wst[:], w4[j])

            # SBUF-only elementwise multiplies (DVE, bf16 2x)
            y0 = ypool.tile([128, s], BF16, tag="y0")
            nc.vector.tensor_mul(y0[:], x4[:], kw1t0_sb[:])
            y1 = ypool.tile([128, s], BF16, tag="y1")
            nc.vector.tensor_mul(y1[:], x4[:], kw1t1_sb[:])
            kdm = zpool.tile([128, s], BF16, tag="kdm")
            nc.vector.tensor_mul(kdm[:], x4[:], kdt_sb[:])

            Bs0 = bspool.tile([128, s], BF16, tag="bs0")
            Bs1 = bspool.tile([128, s], BF16, tag="bs1")
            z0 = zpool.tile([128, s], BF16, tag="z0")
            z1 = zpool.tile([128, s], BF16, tag="z1")
            o = opool.tile([128, s], BF16, tag="o")

            As = []
            for h in range(2):
                hs = bass.ts(h, half)

                # reduce+broadcast matmuls (stationary: ones)
                B0 = psB0.tile([128, half], F32, tag="B0")
                for c in range(nch):
                    cs = bass.ts(h * nch + c, ch)
                    nc.tensor.matmul(B0[:, bass.ts(c, ch)], ones_sb[:],
                                     y0[:, cs], start=True, stop=True)
                B1 = psB1.tile([128, half], F32, tag="B1")
                for c in range(nch):
                    cs = bass.ts(h * nch + c, ch)
                    nc.tensor.matmul(B1[:, bass.ts(c, ch)], ones_sb[:],
                                     y1[:, cs], start=True, stop=True)

                # evacuate broadcasts (ACT), then multiply by kw2 (DVE)
                nc.scalar.copy(Bs0[:, hs], B0[:])
                nc.scalar.copy(Bs1[:, hs], B1[:])
                nc.vector.tensor_mul(z0[:, hs], Bs0[:, hs], kw2t0_sb[:, hs])
                nc.vector.tensor_mul(z1[:, hs], Bs1[:, hs], kw2t1_sb[:, hs])

                # correction matmuls (stationary: wst) open the psum groups
                A = psA.tile([128, half], F32, tag="A")
                As.append(A)
                for c in range(nch):
                    cs = bass.ts(h * nch + c, ch)
                    nc.tensor.matmul(A[:, bass.ts(c, ch)], wst[:], x4[:, cs],
                                     start=True, stop=False)

            # identity-accumulates (stationary: ident), grouped late so z's
            # are ready; banks interleave but each bank's group is ordered
            for h in range(2):
                A = As[h]
                for zt in (z0, z1):
                    for c in range(nch):
                        cs = bass.ts(h * nch + c, ch)
                        nc.tensor.matmul(A[:, bass.ts(c, ch)], ident_sb[:],
                                         zt[:, cs], start=False, stop=False)
                for c in range(nch):
                    cs = bass.ts(h * nch + c, ch)
                    nc.tensor.matmul(A[:, bass.ts(c, ch)], ident_sb[:],
                                     kdm[:, cs], start=False, stop=True)
                nc.scalar.copy(o[:, bass.ts(h, half)], A[:])

            nc.sync.dma_start(out4[j], o[:])

    nc.compile()
    return nc


def build_nc_v4(nj=NJ, s=S, ch=CH):
    """v4: like v3 plus (a) j-pairs share one (128, 2s) x tile so the
    y0/y1/kdm DVE multiplies run at double width (less per-op overhead),
    (b) B0|B1 share one 4-bank psum tile so each half needs a single
    2048-wide ACT evacuation instead of two."""
    nc = bacc.Bacc("TRN2", target_bir_lowering=False)

    half = s // 2
    ch = min(ch, half)
    nch = half // ch
    assert nj % 2 == 0

    xs = nc.dram_tensor("xs", (nj, 128, s), BF16, kind="ExternalInput")
    w4 = nc.dram_tensor("w4", (nj, 128, 128), BF16, kind="ExternalInput")
    onesb = nc.dram_tensor("onesb", (128, 128), BF16, kind="ExternalInput")
    identb = nc.dram_tensor("identb", (128, 128), BF16, kind="ExternalInput")
    kw1t0 = nc.dram_tensor("kw1t0", (128, s), BF16, kind="ExternalInput")
    kw1t1 = nc.dram_tensor("kw1t1", (128, s), BF16, kind="ExternalInput")
    kw2t0 = nc.dram_tensor("kw2t0", (128, s), BF16, kind="ExternalInput")
    kw2t1 = nc.dram_tensor("kw2t1", (128, s), BF16, kind="ExternalInput")
    kdt = nc.dram_tensor("kdt", (128, s), BF16, kind="ExternalInput")
    out4 = nc.dram_tensor("out4", (nj, 128, s), BF16, kind="ExternalOutput")

    with tile.TileContext(nc) as tc, ExitStack() as ctx:
        # persistent weights; kw1/kdt doubled along free for 2s-wide ops
        wpool = ctx.enter_context(tc.tile_pool(name="weights", bufs=1))
        ones_sb = wpool.tile([128, 128], BF16, tag="ones")
        nc.sync.dma_start(ones_sb[:], onesb[:])
        ident_sb = wpool.tile([128, 128], BF16, tag="ident")
        nc.sync.dma_start(ident_sb[:], identb[:])
        kw1t0_sb = wpool.tile([128, 2 * s], BF16, tag="kw1t0")
        nc.sync.dma_start(kw1t0_sb[:, 0:s], kw1t0[:])
        nc.sync.dma_start(kw1t0_sb[:, s:2 * s], kw1t0[:])
        kw1t1_sb = wpool.tile([128, 2 * s], BF16, tag="kw1t1")
        nc.sync.dma_start(kw1t1_sb[:, 0:s], kw1t1[:])
        nc.sync.dma_start(kw1t1_sb[:, s:2 * s], kw1t1[:])
        kdt_sb = wpool.tile([128, 2 * s], BF16, tag="kdt")
        nc.sync.dma_start(kdt_sb[:, 0:s], kdt[:])
        nc.sync.dma_start(kdt_sb[:, s:2 * s], kdt[:])
        kw2t0_sb = wpool.tile([128, s], BF16, tag="kw2t0")
        nc.sync.dma_start(kw2t0_sb[:], kw2t0[:])
        kw2t1_sb = wpool.tile([128, s], BF16, tag="kw2t1")
        nc.sync.dma_start(kw2t1_sb[:], kw2t1[:])

        xpool = ctx.enter_context(tc.tile_pool(name="x", bufs=2))
        wstp = ctx.enter_context(tc.tile_pool(name="wst", bufs=2))
        ypool = ctx.enter_context(tc.tile_pool(name="y", bufs=2))
        kpool = ctx.enter_context(tc.tile_pool(name="k", bufs=2))
        zpool = ctx.enter_context(tc.tile_pool(name="z", bufs=2))
        bspool = ctx.enter_context(tc.tile_pool(name="bs", bufs=3))
        opool = ctx.enter_context(tc.tile_pool(name="o", bufs=2))
        psA = ctx.enter_context(tc.tile_pool(name="psA", bufs=2, space="PSUM"))
        psB = ctx.enter_context(tc.tile_pool(name="psB", bufs=1, space="PSUM"))

        for jp in range(nj // 2):
            j0 = 2 * jp
            x2 = xpool.tile([128, 2 * s], BF16, tag="x2")
            nc.sync.dma_start(x2[:, 0:s], xs[j0])
            nc.sync.dma_start(x2[:, s:2 * s], xs[j0 + 1])
            wst2 = wstp.tile([128, 256], BF16, tag="wst2")
            nc.sync.dma_start(wst2[:, 0:128], w4[j0])
            nc.sync.dma_start(wst2[:, 128:256], w4[j0 + 1])

            # double-width elementwise multiplies (DVE bf16 2x)
            y0 = ypool.tile([128, 2 * s], BF16, tag="y0")
            nc.vector.tensor_mul(y0[:], x2[:], kw1t0_sb[:])
            y1 = ypool.tile([128, 2 * s], BF16, tag="y1")
            nc.vector.tensor_mul(y1[:], x2[:], kw1t1_sb[:])
            kdm = kpool.tile([128, 2 * s], BF16, tag="kdm")
            nc.vector.tensor_mul(kdm[:], x2[:], kdt_sb[:])

            o = opool.tile([128, 2 * s], BF16, tag="o")

            for jj in range(2):
                xoff = jj * s
                z0 = zpool.tile([128, s], BF16, tag="z0")
                z1 = zpool.tile([128, s], BF16, tag="z1")

                As = []
                for h in range(2):
                    hoff = xoff + h * half
                    hs = bass.ts(h, half)

                    # reduce+broadcast for both i into one 4-bank psum tile
                    Bb = psB.tile([128, 2 * half], F32, tag="B")
                    for c in range(nch):
                        nc.tensor.matmul(
                            Bb[:, bass.ts(c, ch)], ones_sb[:],
                            y0[:, hoff:hoff + half][:, bass.ts(c, ch)],
                            start=True, stop=True)
                    for c in range(nch):
                        nc.tensor.matmul(
                            Bb[:, bass.ts(nch + c, ch)], ones_sb[:],
                            y1[:, hoff:hoff + half][:, bass.ts(c, ch)],
                            start=True, stop=True)

                    # one wide evac of [B0h | B1h], then the two z mults
                    Bs = bspool.tile([128, 2 * half], BF16, tag="bs")
                    nc.scalar.copy(Bs[:], Bb[:])
                    nc.vector.tensor_mul(z0[:, hs], Bs[:, 0:half],
                                         kw2t0_sb[:, hs])
                    nc.vector.tensor_mul(z1[:, hs], Bs[:, half:2 * half],
                                         kw2t1_sb[:, hs])

                    # correction matmuls open the A psum groups
                    A = psA.tile([128, half], F32, tag="A")
                    As.append(A)
                    for c in range(nch):
                        nc.tensor.matmul(
                            A[:, bass.ts(c, ch)], wst2[:, bass.ts(jj, 128)],
                            x2[:, hoff:hoff + half][:, bass.ts(c, ch)],
                            start=True, stop=False)

                # identity-accumulates, then evacuate
                for h in range(2):
                    hoff = xoff + h * half
                    A = As[h]
                    for zt in (z0, z1):
                        for c in range(nch):
                            nc.tensor.matmul(
                                A[:, bass.ts(c, ch)], ident_sb[:],
                                zt[:, h * half:(h + 1) * half][:, bass.ts(c, ch)],
                                start=False, stop=False)
                    for c in range(nch):
                        nc.tensor.matmul(
                            A[:, bass.ts(c, ch)], ident_sb[:],
                            kdm[:, hoff:hoff + half][:, bass.ts(c, ch)],
                            start=False, stop=True)
                    nc.scalar.copy(o[:, hoff:hoff + half], A[:])

            nc.sync.dma_start(out4[j0], o[:, 0:s])
            nc.sync.dma_start(out4[j0 + 1], o[:, s:2 * s])

    nc.compile()
    return nc


def prep_core_inputs_v2(xc, w, qw1c, qw2c, kw1, kw2, qddc, kdd):
    """v2 per-core input map: bf16, correction-only W' (no identity)."""
    import ml_dtypes

    bf16 = ml_dtypes.bfloat16
    mm, tc_, s = xc.shape
    nj = tc_ // TB

    xs = np.ascontiguousarray(
        xc.reshape(mm, nj, TB, s).transpose(1, 2, 0, 3).reshape(nj, TB * mm, s)
    ).astype(bf16)

    # per-t correction matrix W'_t[m, n] (no identity)
    wt = np.broadcast_to(w.astype(np.float32), (tc_, mm, mm)).copy()
    wt += np.einsum("tmi,tni->tmn", qw1c, qw2c, dtype=np.float64).astype(np.float32)
    idx = np.arange(mm)
    wt[:, idx, idx] += qddc
    w4 = np.zeros((nj, TB, mm, TB, mm), dtype=np.float32)
    tb = np.arange(TB)
    w4[:, tb, :, tb, :] = wt.reshape(nj, TB, mm, mm).transpose(1, 0, 2, 3)
    w4 = w4.reshape(nj, TB * mm, TB * mm).astype(bf16)

    onesb = np.zeros((TB, mm, TB, mm), dtype=np.float32)
    onesb[tb, :, tb, :] = 1.0
    onesb = onesb.reshape(TB * mm, TB * mm).astype(bf16)
    identb = np.eye(TB * mm, dtype=np.float32).astype(bf16)

    def tile4(a):  # (S, M) -> (128, S) bf16
        return np.ascontiguousarray(np.tile(a.T.astype(np.float32), (TB, 1))).astype(bf16)

    return {
        "xs": xs,
        "w4": w4,
        "onesb": onesb,
        "identb": identb,
        "kw1t0": tile4(kw1[:, :, 0]),
        "kw1t1": tile4(kw1[:, :, 1]),
        "kw2t0": tile4(kw2[:, :, 0]),
        "kw2t1": tile4(kw2[:, :, 1]),
        "kdt": tile4(kdd),
    }


_NC_CACHE = {}

# test-harness knobs (not used by the grading path)
TRACE = False
TRACE_DIR = None
LAST_RES = None
VERSION = 3


def _get_nc():
    key = f"nc_v{VERSION}"
    if key not in _NC_CACHE:
        builders = {1: build_nc, 2: build_nc_v2, 3: build_nc_v3, 4: build_nc_v4}
        _NC_CACHE[key] = builders[VERSION]()
    return _NC_CACHE[key]


def kernel(inputs, w, qw1, qw2, kw1, kw2, qdd, kdd):
    from concourse.bass_utils import run_bass_kernel_spmd

    x = np.asarray(inputs)          # (1, 32, T, S)
    w_ = np.asarray(w)[0]           # (M, M)
    qw1_ = np.asarray(qw1)[0, :, 0]  # (T, M, I)
    qw2_ = np.asarray(qw2)[0, :, 0]
    kw1_ = np.asarray(kw1)[0, :, 0]  # (S, M, I)
    kw2_ = np.asarray(kw2)[0, :, 0]
    qdd_ = np.asarray(qdd)[0, :, 0]  # (T, M)
    kdd_ = np.asarray(kdd)[0, :, 0]  # (S, M)

    xm = x[0]  # (M, T, S)

    prep = prep_core_inputs_v2 if VERSION >= 2 else prep_core_inputs
    in_maps = []
    for c in range(NCORES):
        tsl = slice(c * TC, (c + 1) * TC)
        in_maps.append(
            prep(
                xm[:, tsl, :], w_, qw1_[tsl], qw2_[tsl], kw1_, kw2_,
                qdd_[tsl], kdd_,
            )
        )

    nc = _get_nc()
    kwargs = {}
    if TRACE:
        kwargs = {"trace": True, "tmpdir": TRACE_DIR}
    res = run_bass_kernel_spmd(nc, in_maps, list(range(NCORES)), **kwargs)
    global LAST_RES
    LAST_RES = res

    out = np.empty((M, T, S), dtype=np.float32)
    for c in range(NCORES):
        corr = gather_core_output(np.asarray(res.results[c]["out4"], dtype=np.float32))
        out[:, c * TC:(c + 1) * TC, :] = corr
    if VERSION >= 2:
        out += xm  # exact fp32 residual
    return out.reshape(B, H, T, S)
